# revision 52
# baseline (speedup 1.0000x reference)
"""Bass/Trainium2 kernel for a 6-layer GPT-style transformer (BigramLanguageModel).

Contract: kernel(**inputs) takes the FULL unsharded inputs from
reference.setup_inputs() and returns the FULL [32, 512, 65] fp32 logits.

Sharding: data-parallel over batch. Each of the 8 NeuronCores runs the whole
model on 4 of the 32 sequences (params replicated); outputs are concatenated
on the host. No collectives.

Device-side design (per core, 2048 tokens), v7 -- evolved from the v4
baseline (kernel_v4_baseline.py, 1147us) via trace-driven fixes; measures
~1014us (best observed 1014315 ns, rel-err 7.9e-3). The last ~37us came
from: ln_tp PSUM->SBUF copies pinned to ACT (the DVE halves were queuing
behind LN stats chains and, with genT bufs=1, stalled the next chunk's
transposes ~5.8us/layer); ACT-only relu in the tail-hoisted w1_block(0)
(its DVE-half relus queued behind the attention-tail chain and stalled
PSUM rotation); pos_emb added via an accumulated fp32r identity matmul
into the embedding PSUM group instead of 16 serialized gpsimd accum-DMAs
(~19us chain that also gated LN stats); LM-head copies pinned to ACT.
Attempts that were flat or regressed: LN apply on GPSIMD (+730us, gpsimd
elementwise far too slow), LN apply on ACT via Identity(x*rstd-mean*rstd)
(flat), pend-inject after W2 instead of between W1/W2 (+32us). Key additions over the intermediate v5 (~1106us): a ~40-matmul
HAM warmup burst at t=0 (first K=8/8 moves from ~49us to ~15us), the W1
n-block-0 hoisted into the attention tail (its ~8us of dep-free matmuls
cover the end-of-attention DVE backlog that head-blocked proj(12..15)),
attention-top chunk reordering so the pended LN1-g3 transposes sit behind
6 QK chunks + 4 V tiles of cover, and per-tile bn_stats emitted inside the
W2 loop right after each x-add (shrinks the LN tail chain ~2.8us). NB:
emitting per-tile stats between the PROJ tail x-adds regresses ~47us (it
delays the x-adds that free PSUM gen rotation) -- only do it in the W2
loop. Details:
 - LayerNorm transposes run on the PE in bf16 (1 cyc/row vs fp32's 2):
   the LN apply emits bf16 ht tiles, PE-transposed via a bf16 identity into
   a half-bank bf16 PSUM tile. (XBAR dma_start_transpose is far worse:
   ~15 GB/s and the DGE doorbell blocks the issuing engine ~1.3us each.)
 - LN emission is SPLIT: stats+apply (DVE) are emitted right after their
   producer x-tiles finalize (inside the MLP W2 tail / proj tail /
   embedding loop), while the PE transposes are deferred through a pend
   queue and injected behind ready PE work (between W1 and W2 of each MLP
   n-block, before q-chunks 2/3 at attention top, before LM tiles 8/12).
   This keeps the PE FIFO head from blocking on the DVE chain at phase
   boundaries and keeps HAM near K=8/8.
 - attention: score matmuls contract over head_dim=64, so the even-head
   (PE rows 0:63) and odd-head (rows 64:127) units are issued back-to-back
   and run concurrently in distinct PE row-groups (tile_position derived
   from base_partition), halving score streaming and hiding their
   LDWEIGHTS under the other sub's matmul. Units run in 2-unit waves
   (sub0+sub1 of one sequence); exp is batched per (ki, sub) straight out
   of PSUM on ACT; the causal mask is a DVE multiply of the bf16 diagonal
   block by a 0/1 lower-tri mask; V is augmented as [ones64 | V64] so the
   softmax denominator lands in PSUM rows 0:64 (reciprocal_approx_fast
   needs partition-0 input). PSUM: gen(3)+genT(1) on psA, at(2)+ot(2) on
   psB = 8 banks.
 - engine placement is load-balance-tuned and surprisingly sensitive:
   QK-chunk copies DVE-only, V copies ACT, ln_tp/LM copies ACT, relu
   alternates ACT/DVE except ACT-only in the tail-hoisted W1 block.
   Moving proj evictions onto ACT regresses ~200us (ACT queue convoys vs
   the attention exps); GPSIMD elementwise and XBAR-DMA transposes
   regress similarly.
 - attention-tail ordering: proj tiles 0..7 are PE fillers inside pair-2
   waves; the tail emits proj(8..11), LN2-g1 stats, tp(g0), proj(12..15),
   LN2-g2 stats, tp(g1), then pends tp(g2)/tp(g3) into the MLP.
 - per-layer weight prefetch order puts the pair-0 QK weight DMAs first;
   w1all/w2all are double-buffered so their WAR on the previous MLP can't
   stall the sync DGE queue ahead of attention.
 - bf16 everywhere on the PE except the fp32 residual and the fp32r
   embedding path; LN gains/biases are folded into weights host-side.
 - do NOT try fp8 DoubleRow here: e4m3 quantization of any large matmul
   class pushes rel-err to 6e-2..1.3e-1 vs the 2e-2 gate (bf16 baseline
   noise is already 8.7e-3).
"""

import sys

for _p in ("/opt/trn_rl_repo", "/opt/pypackages"):
    if _p not in sys.path:
        sys.path.insert(0, _p)

import numpy as np
import ml_dtypes

import concourse.bass as bass
import concourse.tile as tile
from concourse import bacc, mybir
from concourse.bass_utils import run_bass_kernel_spmd

F32 = mybir.dt.float32
F32R = mybir.dt.float32r
BF16 = mybir.dt.bfloat16

N_EMBED = 384
CONTEXT = 512
N_HEADS = 6
HEAD_DIM = 64
N_LAYERS = 6
VOCAB = 65
B, T = 32, 512
LN_EPS = 1e-5
N_CORES = 8
B_LOC = B // N_CORES          # 4 sequences per core
N_TOK = B_LOC * T             # 2048 tokens per core
N_TILES = N_TOK // 128        # 16 token tiles
N_CHUNKS = N_EMBED // 128     # 3 E-chunks
N_MLP = 4 * N_EMBED           # 1536
N_MCHUNK = N_MLP // 128       # 12
SCALE = float(N_EMBED) ** -0.5
MDT = F32R
NEG_BIG = -1.0e30
V_W = N_HEADS * 128           # [ones64 | V64] per head -> 768 cols


def _prep(inputs):
    """Host-side layout prep + exact LN folds. Returns (shared, has, per_core_idx)."""
    f = lambda a: np.ascontiguousarray(np.asarray(a), dtype=np.float32)
    idx = np.asarray(inputs["idx"])
    tok_emb, pos_emb = f(inputs["tok_emb"]), f(inputs["pos_emb"])
    Wq, Wk, Wv = f(inputs["Wq"]), f(inputs["Wk"]), f(inputs["Wv"])
    Wproj, bproj = f(inputs["Wproj"]), f(inputs["bproj"])
    W1, b1, W2, b2 = f(inputs["W1"]), f(inputs["b1"]), f(inputs["W2"]), f(inputs["b2"])
    ln1_g, ln1_b = f(inputs["ln1_g"]), f(inputs["ln1_b"])
    ln2_g, ln2_b = f(inputs["ln2_g"]), f(inputs["ln2_b"])
    lnf_g, lnf_b = f(inputs["lnf_g"]), f(inputs["lnf_b"])
    Wlm, blm = f(inputs["Wlm"]), f(inputs["blm"])

    L, H, E, D = N_LAYERS, N_HEADS, N_EMBED, HEAD_DIM

    # fold ln gains into the consuming weights (exact)
    Wq_f = ln1_g[:, None, :, None] * Wq          # [L,H,E,D]
    Wk_f = ln1_g[:, None, :, None] * Wk
    Wv_f = ln1_g[:, None, :, None] * Wv
    W1_f = ln2_g[:, :, None] * W1                # [L,E,4E]
    Wlm_f = lnf_g[:, None] * Wlm                 # [E,V]

    # ln biases propagate through the matmuls as constant bias vectors
    qb = np.einsum("le,lhed->lhd", ln1_b, Wq)    # [L,H,D]
    kb = np.einsum("le,lhed->lhd", ln1_b, Wk)
    vb = np.einsum("le,lhed->lhd", ln1_b, Wv)
    b1_eff = b1 + np.einsum("le,lem->lm", ln2_b, W1)    # [L,4E]
    blm_eff = blm + lnf_b @ Wlm                          # [V]

    # head-pair packed QT/KT weights: [L, 3, E, 128]  (pair r = heads 2r, 2r+1)
    wqp = np.concatenate([Wq_f[:, 0::2], Wq_f[:, 1::2]], axis=-1)  # [L,3,E,128]
    wkp = np.concatenate([Wk_f[:, 0::2], Wk_f[:, 1::2]], axis=-1)
    qbp = np.concatenate([qb[:, 0::2], qb[:, 1::2]], axis=-1)      # [L,3,128]
    kbp = np.concatenate([kb[:, 0::2], kb[:, 1::2]], axis=-1)
    wv_all = Wv_f.transpose(0, 2, 1, 3).reshape(L, E, H * D)       # [L,E,384]
    vb_all = vb.reshape(L, H * D)

    # causal mask as additive matmul rhs: -BIG on strict lower triangle (k > j)
    trib = (np.tril(np.ones((128, 128), dtype=np.float32), -1) * NEG_BIG)

    shared = dict(
        tok_emb=tok_emb,
        pos_emb=pos_emb,
        wqp=np.ascontiguousarray(wqp.astype(ml_dtypes.bfloat16)),
        wkp=np.ascontiguousarray(wkp.astype(ml_dtypes.bfloat16)),
        wv=np.ascontiguousarray(wv_all.astype(ml_dtypes.bfloat16)),
        wp=np.ascontiguousarray(Wproj.astype(ml_dtypes.bfloat16)),
        w1=np.ascontiguousarray(W1_f.astype(ml_dtypes.bfloat16)),
        w2=np.ascontiguousarray(W2.astype(ml_dtypes.bfloat16)),
        wlm=np.ascontiguousarray(Wlm_f.astype(ml_dtypes.bfloat16)),
        ident=np.eye(128, dtype=ml_dtypes.bfloat16),
        identf=np.eye(128, dtype=np.float32),
        pos4=np.ascontiguousarray(
            pos_emb[:CONTEXT].reshape(4, 128, N_EMBED)),
        iota=np.arange(VOCAB, dtype=np.float32).reshape(VOCAB, 1),
        trib=np.ascontiguousarray(trib.astype(ml_dtypes.bfloat16)),
    )
    flags = dict(
        qb=qbp if np.any(qbp) else None,
        kb=kbp if np.any(kbp) else None,
        vb=np.broadcast_to(vb_all[:, None, :], (L, 128, H * D)).copy()
        if np.any(vb) else None,
        bp=np.broadcast_to(bproj[:, None, :], (L, 128, E)).copy()
        if np.any(bproj) else None,
        b1=np.ascontiguousarray(b1_eff.reshape(L, N_MCHUNK, 128).transpose(0, 2, 1))
        if np.any(b1_eff) else None,                    # [L,128,12] partition-major
        b2=np.broadcast_to(b2[:, None, :], (L, 128, E)).copy() if np.any(b2) else None,
        blm=np.broadcast_to(blm_eff[None, :], (128, VOCAB)).copy()
        if np.any(blm_eff) else None,
    )
    for k, v in flags.items():
        if v is not None:
            shared[k] = np.ascontiguousarray(v, dtype=np.float32)
    has = {k: (v is not None) for k, v in flags.items()}

    idx_f = idx.astype(np.float32).reshape(N_CORES, N_TOK)
    return shared, has, idx_f


def _build(has):
    nc = bacc.Bacc(trn_type="TRN2", debug=False, num_devices=N_CORES)
    d = {}
    d["idxf"] = nc.dram_tensor("idxf", [N_TOK], F32, kind="ExternalInput")
    d["tok_emb"] = nc.dram_tensor("tok_emb", [VOCAB, N_EMBED], MDT, kind="ExternalInput")
    d["pos_emb"] = nc.dram_tensor("pos_emb", [CONTEXT, N_EMBED], F32, kind="ExternalInput")
    d["wqp"] = nc.dram_tensor("wqp", [N_LAYERS, 3, N_EMBED, 128], BF16, kind="ExternalInput")
    d["wkp"] = nc.dram_tensor("wkp", [N_LAYERS, 3, N_EMBED, 128], BF16, kind="ExternalInput")
    d["wv"] = nc.dram_tensor("wv", [N_LAYERS, N_EMBED, N_EMBED], BF16, kind="ExternalInput")
    d["wp"] = nc.dram_tensor("wp", [N_LAYERS, N_EMBED, N_EMBED], BF16, kind="ExternalInput")
    d["w1"] = nc.dram_tensor("w1", [N_LAYERS, N_EMBED, N_MLP], BF16, kind="ExternalInput")
    d["w2"] = nc.dram_tensor("w2", [N_LAYERS, N_MLP, N_EMBED], BF16, kind="ExternalInput")
    d["wlm"] = nc.dram_tensor("wlm", [N_EMBED, VOCAB], BF16, kind="ExternalInput")
    d["ident"] = nc.dram_tensor("ident", [128, 128], BF16, kind="ExternalInput")
    d["identf"] = nc.dram_tensor("identf", [128, 128], MDT, kind="ExternalInput")
    d["pos4"] = nc.dram_tensor("pos4", [4, 128, N_EMBED], MDT, kind="ExternalInput")
    d["iota"] = nc.dram_tensor("iota", [VOCAB, 1], F32, kind="ExternalInput")
    d["trib"] = nc.dram_tensor("trib", [128, 128], BF16, kind="ExternalInput")
    if has["qb"]:
        d["qb"] = nc.dram_tensor("qb", [N_LAYERS, 3, 128], F32, kind="ExternalInput")
    if has["kb"]:
        d["kb"] = nc.dram_tensor("kb", [N_LAYERS, 3, 128], F32, kind="ExternalInput")
    if has["vb"]:
        d["vb"] = nc.dram_tensor("vb", [N_LAYERS, 128, N_EMBED], F32, kind="ExternalInput")
    if has["bp"]:
        d["bp"] = nc.dram_tensor("bp", [N_LAYERS, 128, N_EMBED], F32, kind="ExternalInput")
    if has["b1"]:
        d["b1"] = nc.dram_tensor("b1", [N_LAYERS, 128, N_MCHUNK], F32, kind="ExternalInput")
    if has["b2"]:
        d["b2"] = nc.dram_tensor("b2", [N_LAYERS, 128, N_EMBED], F32, kind="ExternalInput")
    if has["blm"]:
        d["blm"] = nc.dram_tensor("blm", [128, VOCAB], F32, kind="ExternalInput")
    logits_d = nc.dram_tensor("logits", [N_TOK, VOCAB], F32, kind="ExternalOutput")

    AF = mybir.ActivationFunctionType
    OP = mybir.AluOpType

    with tile.TileContext(nc) as tc:
        with tc.tile_pool(name="const", bufs=1) as cst, \
             tc.tile_pool(name="persist", bufs=1) as per, \
             tc.tile_pool(name="work", bufs=3) as wk, \
             tc.tile_pool(name="htile", bufs=4) as hp, \
             tc.tile_pool(name="wts", bufs=4) as wts, \
             tc.tile_pool(name="psA", bufs=2, space="PSUM") as psA, \
             tc.tile_pool(name="psB", bufs=2, space="PSUM") as psB:

            # ---- constants (embedding-critical DMAs first) ----
            iota = cst.tile([VOCAB, 1], F32)
            nc.sync.dma_start(iota, d["iota"][:, :])
            tok_sb = cst.tile([VOCAB, N_EMBED], MDT)
            nc.sync.dma_start(tok_sb, d["tok_emb"][:, :])
            ident = cst.tile([128, 128], BF16)
            nc.sync.dma_start(ident, d["ident"][:, :])
            identf = cst.tile([128, 128], MDT)
            nc.sync.dma_start(identf, d["identf"][:, :])
            pos_sb = cst.tile([128, 4, N_EMBED], MDT)
            nc.sync.dma_start(pos_sb, d["pos4"].rearrange("q p e -> p q e"))
            trib = cst.tile([128, 128], BF16)
            nc.sync.dma_start(trib, d["trib"][:, :])
            eps_sb = cst.tile([128, 1], F32)
            nc.vector.memset(eps_sb, LN_EPS)
            tri01 = cst.tile([128, 128], BF16)
            nc.vector.tensor_scalar(out=tri01, in0=trib, scalar1=0.0,
                                    scalar2=None, op0=OP.is_equal)

            bias_sb = {}
            for nm in ("vb", "bp", "b2"):
                if has[nm]:
                    bias_sb[nm] = cst.tile([128, N_LAYERS, N_EMBED], F32)
                    nc.sync.dma_start(bias_sb[nm], d[nm].rearrange("l p e -> p l e"))
            if has["b1"]:
                bias_sb["b1"] = cst.tile([128, N_LAYERS, N_MCHUNK], F32)
                nc.sync.dma_start(bias_sb["b1"], d["b1"].rearrange("l p m -> p l m"))
            for nm in ("qb", "kb"):
                if has[nm]:
                    bias_sb[nm] = cst.tile([128, N_LAYERS, 3], F32)
                    nc.sync.dma_start(bias_sb[nm], d[nm].rearrange("l r p -> p l r"))
            if has["blm"]:
                bias_sb["blm"] = cst.tile([128, VOCAB], F32)
                nc.sync.dma_start(bias_sb["blm"], d["blm"][:, :])

            # ---- HAM warmup: ~40 dep-free matmuls keep the PE busy from
            # t=0 so the clock gate opens at ~4us instead of ~49us (the
            # embedding phase is DMA/DVE-gated and too sparse to warm it) ----
            for _w in range(40):
                pwm = psA.tile([128, 512], F32, tag="gen", name="pwm")
                nc.tensor.matmul(pwm[:, :128], lhsT=ident, rhs=ident,
                                 start=True, stop=True)

            # ---- persistent activations ----
            x = per.tile([128, N_TILES, N_EMBED], F32)          # residual, token-major
            v_aug = per.tile([128, N_TILES, V_W], BF16)         # [ones64|V64] per head
            ones_blk = cst.tile([128, 64], F32)
            nc.vector.memset(ones_blk, 1.0)
            nc.vector.tensor_copy(
                v_aug.rearrange("p t (h j) -> p t h j", h=N_HEADS)[:, :, :, 0:64],
                ones_blk[:, None, None, :].to_broadcast(
                    [128, N_TILES, N_HEADS, 64]))

            # round-robin engine pickers for PSUM->SBUF copies
            _rr = {"c": 0, "q": 0}

            def copy_out(dst, src):
                _rr["c"] += 1
                if _rr["c"] % 2 == 0:
                    nc.scalar.copy(dst, src)
                else:
                    nc.vector.tensor_copy(dst, src)

            # =========================================================
            # LayerNorm: stats (DVE/ACT) -> bf16 apply (DVE) -> bf16 PE
            # transposes -> PSUM -> copy into hT.
            # =========================================================
            def tile_stats(t):
                    st = wk.tile([128, 6], F32, tag="bnst", bufs=8)
                    nc.vector.bn_stats(out=st, in_=x[:, t, :])
                    return st

            def ln_stats_apply(nm, tg, sts=None):
                    mv4 = wk.tile([128, 4, 2], F32, tag="mv" + nm)
                    for dt_ in range(4):
                        st = (sts[dt_] if sts is not None
                              else tile_stats(tg * 4 + dt_))
                        nc.vector.bn_aggr(out=mv4[:, dt_, :], in_=st)
                    sstd = wk.tile([128, 4], F32, tag="sstd")
                    nc.scalar.activation(out=sstd, in_=mv4[:, :, 1],
                                         func=AF.Sqrt, bias=eps_sb, scale=1.0)
                    rstd = wk.tile([128, 4], F32, tag="rstd")
                    nc.vector.reciprocal(out=rstd, in_=sstd)
                    hts = []
                    for dt_ in range(4):
                        t = tg * 4 + dt_
                        ht = hp.tile([128, N_EMBED], BF16, tag="h", bufs=12)
                        nc.vector.tensor_scalar(
                            out=ht, in0=x[:, t, :],
                            scalar1=mv4[:, dt_, 0:1],
                            scalar2=rstd[:, dt_:dt_ + 1],
                            op0=OP.subtract, op1=OP.mult)
                        hts.append(ht)
                    return hts

            def ln_tp(dst_hT, hts, tg, act=True):
                    for c in range(N_CHUNKS):
                        pt = psA.tile([128, 512], BF16, tag="genT", bufs=2,
                                      name="pt")
                        for dt_ in range(4):
                            nc.tensor.transpose(
                                pt[:, dt_ * 128:(dt_ + 1) * 128],
                                hts[dt_][:, c * 128:(c + 1) * 128], ident)
                        dst = dst_hT[:, c, tg * 512:(tg + 1) * 512]
                        if act:
                            nc.scalar.copy(dst, pt)
                        else:
                            copy_out(dst, pt)

            def new_hT():
                return per.tile([128, N_CHUNKS, N_TOK], BF16, tag="ht", bufs=2,
                                name="hT")

            # =========================================================
            # embedding: x = onehot(idx) @ tok_emb + pos; LN1 of layer 0
            # =========================================================
            hT = new_hT()
            pend_tp = []
            idx_bs = []
            for t in range(N_TILES):
                idx_b = wk.tile([VOCAB, 128], F32, tag="idxb", bufs=8,
                                name="idx_b")
                nc.sync.dma_start(
                    idx_b,
                    bass.AP(tensor=d["idxf"], offset=t * 128,
                            ap=[[0, VOCAB], [1, 128]]))
                idx_bs.append(idx_b)
            for t in range(N_TILES):
                idx_b = idx_bs[t]
                oh = wk.tile([VOCAB, 128], MDT, tag="oh")
                nc.vector.tensor_scalar(out=oh, in0=idx_b, scalar1=iota,
                                        scalar2=None, op0=OP.is_equal)
                pe = psA.tile([128, 512], F32, tag="gen")
                nc.tensor.matmul(pe[:, :N_EMBED], lhsT=oh,
                                 rhs=tok_sb, start=True, stop=False)
                nc.tensor.matmul(pe[:, :N_EMBED], lhsT=identf,
                                 rhs=pos_sb[:, t % 4, :],
                                 start=False, stop=True)
                nc.scalar.copy(x[:, t, :], pe[:, :N_EMBED])
                if t % 4 == 3:
                    if pend_tp:
                        pend_tp.pop(0)()
                    hts_g = ln_stats_apply("1", t // 4)
                    pend_tp.append(lambda hts=hts_g, g=t // 4, dst=hT:
                                   ln_tp(dst, hts, g))

            for layer in range(N_LAYERS):
                # ---- prefetch all weights of this layer (pair-0 QK weight
                # DMAs must go FIRST: w1all/w2all WAR-wait on the previous
                # MLP inside the same sync DGE queue and would stall the
                # first attention matmuls behind them) ----
                otc = per.tile([128, N_CHUNKS, N_TOK], BF16, tag="big")

                # ---- QT/KT chunk emitters ----
                def emit_qk_chunks(pair, halves=False):
                    qkt, chunks = {}, []
                    W = 256 if halves else 512
                    for nm, wd, bias_nm in (("q", d["wqp"], "qb"),
                                            ("k", d["wkp"], "kb")):
                        wqk = wts.tile([128, N_CHUNKS, 128], BF16, tag="wqk",
                                       bufs=2, name=f"wqk_{nm}")
                        for c in range(N_CHUNKS):
                            nc.sync.dma_start(
                                wqk[:, c, :],
                                wd[layer, pair, c * 128:(c + 1) * 128, :])
                        # rows 0:64 = even head (sub0), 64:128 = odd head
                        dstT = per.tile([128, N_TOK], BF16, tag="qk" + nm,
                                        bufs=2, name=f"qk_{nm}")
                        qkt[nm] = dstT

                        def chunk(n, wqk=wqk, dstT=dstT, bias_nm=bias_nm):
                            pq = psA.tile([128, 512], F32, tag="gen", name="pq")
                            for c in range(N_CHUNKS):
                                nc.tensor.matmul(
                                    pq[:, :W], lhsT=wqk[:, c, :],
                                    rhs=hT[:, c, n * W:(n + 1) * W],
                                    start=(c == 0), stop=(c == N_CHUNKS - 1))
                            dst = dstT[:, n * W:(n + 1) * W]
                            if has[bias_nm]:
                                nc.scalar.activation(
                                    out=dst, in_=pq[:, :W], func=AF.Identity,
                                    bias=bias_sb[bias_nm][:, layer, pair:pair + 1],
                                    scale=1.0)
                            else:
                                nc.vector.tensor_copy(dst, pq[:, :W])

                        for n in range(N_TOK // W):
                            chunks.append(lambda n=n, chunk=chunk: chunk(n))
                    return qkt, chunks

                # ---- V tile emitter ----
                def emit_v(t):
                    pv = psA.tile([128, 512], F32, tag="gen")
                    for c in range(N_CHUNKS):
                        nc.tensor.matmul(pv[:, :N_EMBED],
                                         lhsT=hT[:, c, t * 128:(t + 1) * 128],
                                         rhs=wv_c[c],
                                         start=(c == 0), stop=(c == N_CHUNKS - 1))
                    src = pv[:, :N_EMBED].rearrange("p (h j) -> p h j", h=N_HEADS)
                    dst = v_aug[:, t, :].rearrange(
                        "p (h j) -> p h j", h=N_HEADS)[:, :, 64:128]
                    if has["vb"]:
                        nc.vector.tensor_tensor(
                            out=dst, in0=src,
                            in1=bias_sb["vb"][:, layer, :].rearrange(
                                "p (h j) -> p h j", h=N_HEADS),
                            op=OP.add)
                    else:
                        nc.scalar.copy(dst, src)

                # ---- proj tile emitter (split=True frees the PSUM buffer
                # via a fast ACT copy; only safe outside the exp-heavy waves)
                def emit_proj(t, split=False):
                    pp = psA.tile([128, 512], F32, tag="gen")
                    for c in range(N_CHUNKS):
                        nc.tensor.matmul(pp[:, :N_EMBED],
                                         lhsT=otc[:, c, t * 128:(t + 1) * 128],
                                         rhs=wp_c[c],
                                         start=(c == 0), stop=(c == N_CHUNKS - 1))
                    src_ap = pp[:, :N_EMBED]
                    if has["bp"]:
                        tmp = hp.tile([128, N_EMBED], F32, tag="hbp")
                        nc.vector.tensor_tensor(out=tmp, in0=src_ap,
                                                in1=bias_sb["bp"][:, layer, :],
                                                op=OP.add)
                        src_ap = tmp
                    if split:
                        po = hp.tile([128, N_EMBED], F32, tag="po", bufs=4)
                        nc.scalar.copy(po, src_ap)
                        src_ap = po
                    nc.vector.tensor_tensor(out=x[:, t, :], in0=src_ap,
                                            in1=x[:, t, :], op=OP.add)

                # ---- attention ----
                qkt, chunks0 = emit_qk_chunks(0)
                wv_c, wp_c = [], []
                for c in range(N_CHUNKS):
                    w = wts.tile([128, N_EMBED], BF16, tag="wvchk", bufs=3)
                    nc.sync.dma_start(w, d["wv"][layer, c * 128:(c + 1) * 128, :])
                    wv_c.append(w)
                for c in range(N_CHUNKS):
                    w = wts.tile([128, N_EMBED], BF16, tag="wpchk", bufs=3)
                    nc.sync.dma_start(w, d["wp"][layer, c * 128:(c + 1) * 128, :])
                    wp_c.append(w)
                w1all = wts.tile([128, N_CHUNKS, N_MLP], BF16, tag="w1all", bufs=2)
                for c in range(N_CHUNKS):
                    nc.sync.dma_start(
                        w1all[:, c, :], d["w1"][layer, c * 128:(c + 1) * 128, :])
                w2all = wts.tile([128, N_MCHUNK, N_EMBED], BF16, tag="w2all", bufs=1)
                for m in range(N_MCHUNK):
                    nc.sync.dma_start(
                        w2all[:, m, :], d["w2"][layer, m * 128:(m + 1) * 128, :])
                # q0 q1 k0 k1 first; tp(ln1-g2) before q2/k2; tp(ln1-g3)
                # deferred behind 4 V tiles so its DVE apply chain (which
                # only starts after W2(3)) is fully covered by PE work
                for i in (0, 1, 4, 5):
                    chunks0[i]()
                if pend_tp:
                    pend_tp.pop(0)()
                for i in (2, 6):
                    chunks0[i]()
                for t in range(4):          # V tiles 0..3 need hT groups 0,1
                    emit_v(t)
                while pend_tp:
                    pend_tp.pop(0)()
                for i in (3, 7):
                    chunks0[i]()
                for t in range(4, 8):
                    emit_v(t)

                fillers = []

                def pop_filler():
                    if fillers:
                        fillers.pop(0)()

                for pair in range(3):
                    if pair == 0:
                        fillers = [lambda t=t: emit_v(t) for t in range(8, 16)]
                        qkt_n, chunks_n = emit_qk_chunks(1)
                        fillers += chunks_n
                    elif pair == 1:
                        qkt_n, chunks_n = emit_qk_chunks(2, halves=True)
                        fillers = list(chunks_n)
                    else:
                        fillers = []

                    # waves: one sequence s, both head-parities (sub 0 on PE
                    # rows 0:63, sub 1 on rows 64:127 -> concurrent scores
                    # into the two banks of one [128,1024] PSUM tile, so the
                    # exp is a single batched ACT op per key tile.
                    wave_order = (0, 1, 2, 3)
                    for wi, s in enumerate(wave_order):
                        pos = [psB.tile([128, 512], F32, tag="ot", bufs=2,
                                        name=f"pos_{s}_{sub}")
                               for sub in range(2)]
                        at = wk.tile([128, 4, 2, 512], BF16, tag="at_sb",
                                     bufs=2, name=f"at_{s}")
                        for ki in range(4):
                            width = 512 - ki * 128
                            kc = s * 512 + ki * 128
                            pa = [psB.tile([128, 512], F32, tag="at",
                                           name=f"pa_{s}_{ki}_{sub}")
                                  for sub in range(2)]
                            for sub in range(2):
                                nc.tensor.matmul(
                                    pa[sub][:, :width],
                                    lhsT=qkt["k"][64 * sub:64 * sub + 64,
                                                  kc:kc + 128],
                                    rhs=qkt["q"][64 * sub:64 * sub + 64,
                                                 kc:s * 512 + 512],
                                    start=True, stop=True)
                            for sub in range(2):
                                nc.scalar.activation(
                                    out=at[:, ki, sub, :width],
                                    in_=pa[sub][:, :width],
                                    func=AF.Exp, scale=SCALE)
                            nc.vector.tensor_tensor(
                                out=at[:, ki, :, 0:128],
                                in0=at[:, ki, :, 0:128],
                                in1=tri01[:, None, :].to_broadcast(
                                    [128, 2, 128]),
                                op=OP.mult)
                            pop_filler()
                            for sub in range(2):
                                h = 2 * pair + sub
                                nc.tensor.matmul(
                                    pos[sub][:, ki * 128:512],
                                    lhsT=v_aug[:, s * 4 + ki,
                                               h * 128:(h + 1) * 128],
                                    rhs=at[:, ki, sub, :width],
                                    start=(ki == 0), stop=(ki == 3))
                        # denominator is always rows 0:64 ([ones|V] layout)
                        rho = wk.tile([64, 2, 512], F32, tag="rho", bufs=2,
                                      name=f"rho_{s}")
                        for sub in range(2):
                            nc.vector.reciprocal_approx_fast(
                                out=rho[:, sub, :], in_=pos[sub][0:64, :])
                            nc.vector.tensor_tensor(
                                out=otc[64 * sub:64 * sub + 64, pair,
                                        s * 512:(s + 1) * 512],
                                in0=pos[sub][64:128, :], in1=rho[:, sub, :],
                                op=OP.mult)
                        if pair == 2 and wi == 1:
                            fillers += [lambda t=t: emit_proj(t)
                                        for t in range(8)]
                        if pair == 2 and wi == 2:
                            hts2_0 = ln_stats_apply("2", 0)
                    # drain fillers before the next pair needs its QT/KT
                    while fillers:
                        pop_filler()
                    if pair < 2:
                        qkt = qkt_n

                # ---- LN2 with split emission: stats/apply go to the DVE
                # queue early, transposes are placed behind ready PE work so
                # the PE FIFO head never waits on the DVE chain ----
                h2T = new_hT()
                mlpT = per.tile([128, N_MCHUNK, 512], BF16, tag="mlpt")

                def w1_block(n, relu_act=False):
                    for m in range(N_MCHUNK):
                        pm = psA.tile([128, 512], F32, tag="gen", name="pm")
                        for c in range(N_CHUNKS):
                            nc.tensor.matmul(
                                pm, lhsT=w1all[:, c, m * 128:(m + 1) * 128],
                                rhs=h2T[:, c, n * 512:(n + 1) * 512],
                                start=(c == 0), stop=(c == N_CHUNKS - 1))
                        if has["b1"]:
                            nc.scalar.activation(
                                out=mlpT[:, m, :], in_=pm, func=AF.Relu,
                                bias=bias_sb["b1"][:, layer, m:m + 1], scale=1.0)
                        elif relu_act or m % 2 == 0:
                            nc.scalar.activation(out=mlpT[:, m, :], in_=pm,
                                                 func=AF.Relu, scale=1.0)
                        else:
                            nc.vector.tensor_scalar(
                                out=mlpT[:, m, :], in0=pm, scalar1=0.0,
                                scalar2=None, op0=OP.max)

                for t in range(8, 11):
                    emit_proj(t)
                hts2_1 = ln_stats_apply("2", 1)
                ln_tp(h2T, hts2_0, 0)
                # W1 of n-block 0: ~8us of dep-free PE work that hides the
                # end-of-attention DVE backlog (norm(s3) + x-adds + stats);
                # proj(11) is deferred past it so its PSUM-buffer WAR on
                # xadd(8) never reaches the PE FIFO head
                w1_block(0, relu_act=True)
                for t in range(11, 16):
                    emit_proj(t)
                hts2_2 = ln_stats_apply("2", 2)
                ln_tp(h2T, hts2_1, 1)
                hts2_3 = ln_stats_apply("2", 3)
                pend_tp.append(lambda hts=hts2_2, dst=h2T: ln_tp(dst, hts, 2))
                pend_tp.append(lambda hts=hts2_3, dst=h2T: ln_tp(dst, hts, 3))

                # ---- MLP, with the next LN1 interleaved in the W2 tail ----
                last = layer == N_LAYERS - 1
                if last:
                    hT = per.tile([128, N_CHUNKS, N_TOK], BF16, tag="xf")
                    wlm_c = []
                    for c in range(N_CHUNKS):
                        w = wts.tile([128, VOCAB], BF16, tag="wlm", bufs=3)
                        nc.sync.dma_start(w, d["wlm"][c * 128:(c + 1) * 128, :])
                        wlm_c.append(w)
                    nm = "f"
                else:
                    hT = new_hT()
                    nm = "1"
                for n in range(N_TOK // 512):
                    if n > 0:
                        w1_block(n)
                    if pend_tp:
                        pend_tp.pop(0)()
                    sts = []
                    for dt in range(4):
                        t = n * 4 + dt
                        pw = psA.tile([128, 512], F32, tag="gen")
                        for m in range(N_MCHUNK):
                            nc.tensor.matmul(
                                pw[:, :N_EMBED],
                                lhsT=mlpT[:, m, dt * 128:(dt + 1) * 128],
                                rhs=w2all[:, m, :],
                                start=(m == 0), stop=(m == N_MCHUNK - 1))
                        if has["b2"]:
                            tmp = hp.tile([128, N_EMBED], F32, tag="hbp")
                            nc.vector.tensor_tensor(out=tmp, in0=pw[:, :N_EMBED],
                                                    in1=bias_sb["b2"][:, layer, :],
                                                    op=OP.add)
                            nc.vector.tensor_tensor(out=x[:, t, :], in0=tmp,
                                                    in1=x[:, t, :], op=OP.add)
                        else:
                            nc.vector.tensor_tensor(out=x[:, t, :],
                                                    in0=pw[:, :N_EMBED],
                                                    in1=x[:, t, :], op=OP.add)
                        sts.append(tile_stats(t))
                    hts_g = ln_stats_apply(nm, n, sts=sts)
                    pend_tp.append(lambda hts=hts_g, g=n, dst=hT:
                                   ln_tp(dst, hts, g))

            # ---- LM head (final LN already produced hT under tag "xf") ----
            for t in range(N_TILES):
                if t in (8, 12) and pend_tp:
                    pend_tp.pop(0)()
                pl = psA.tile([128, 512], F32, tag="gen")
                for c in range(N_CHUNKS):
                    nc.tensor.matmul(pl[:, :VOCAB],
                                     lhsT=hT[:, c, t * 128:(t + 1) * 128],
                                     rhs=wlm_c[c],
                                     start=(c == 0), stop=(c == N_CHUNKS - 1))
                lg = wk.tile([128, VOCAB], F32, tag="lg")
                if has["blm"]:
                    nc.vector.tensor_tensor(out=lg, in0=pl[:, :VOCAB],
                                            in1=bias_sb["blm"], op=OP.add)
                else:
                    nc.scalar.copy(lg, pl[:, :VOCAB])
                nc.sync.dma_start(logits_d[t * 128:(t + 1) * 128, :], lg)

    nc.compile()
    return nc


_CACHE = {}


def _get_nc(has):
    key = tuple(sorted(has.items()))
    if key not in _CACHE:
        _CACHE[key] = _build(has)
    return _CACHE[key]


def kernel(**inputs):
    shared, has, idx_f = _prep(inputs)
    nc = _get_nc(has)
    in_maps = []
    for core in range(N_CORES):
        m = dict(shared)
        m["idxf"] = idx_f[core]
        in_maps.append(m)
    res = run_bass_kernel_spmd(nc, in_maps, core_ids=list(range(N_CORES)))
    out = np.stack([r["logits"].reshape(B_LOC, T, VOCAB) for r in res.results])
    return out.reshape(B, T, VOCAB)


# revision 53
# speedup vs baseline: 1.0763x; 1.0763x over previous
"""Bass/Trainium2 kernel for a 6-layer GPT-style transformer (BigramLanguageModel).

Contract: kernel(**inputs) takes the FULL unsharded inputs from
reference.setup_inputs() and returns the FULL [32, 512, 65] fp32 logits.

Sharding: data-parallel over batch. Each of the 8 NeuronCores runs the whole
model on 4 of the 32 sequences (params replicated); outputs are concatenated
on the host. No collectives.

Device-side design (per core, 2048 tokens), v7 -- evolved from the v4
baseline (kernel_v4_baseline.py, 1147us) via trace-driven fixes; measures
~1014us (best observed 1014315 ns, rel-err 7.9e-3). The last ~37us came
from: ln_tp PSUM->SBUF copies pinned to ACT (the DVE halves were queuing
behind LN stats chains and, with genT bufs=1, stalled the next chunk's
transposes ~5.8us/layer); ACT-only relu in the tail-hoisted w1_block(0)
(its DVE-half relus queued behind the attention-tail chain and stalled
PSUM rotation); pos_emb added via an accumulated fp32r identity matmul
into the embedding PSUM group instead of 16 serialized gpsimd accum-DMAs
(~19us chain that also gated LN stats); LM-head copies pinned to ACT.
Attempts that were flat or regressed: LN apply on GPSIMD (+730us, gpsimd
elementwise far too slow), LN apply on ACT via Identity(x*rstd-mean*rstd)
(flat), pend-inject after W2 instead of between W1/W2 (+32us). Key additions over the intermediate v5 (~1106us): a ~40-matmul
HAM warmup burst at t=0 (first K=8/8 moves from ~49us to ~15us), the W1
n-block-0 hoisted into the attention tail (its ~8us of dep-free matmuls
cover the end-of-attention DVE backlog that head-blocked proj(12..15)),
attention-top chunk reordering so the pended LN1-g3 transposes sit behind
6 QK chunks + 4 V tiles of cover, and per-tile bn_stats emitted inside the
W2 loop right after each x-add (shrinks the LN tail chain ~2.8us). NB:
emitting per-tile stats between the PROJ tail x-adds regresses ~47us (it
delays the x-adds that free PSUM gen rotation) -- only do it in the W2
loop. Details:
 - LayerNorm transposes run on the PE in bf16 (1 cyc/row vs fp32's 2):
   the LN apply emits bf16 ht tiles, PE-transposed via a bf16 identity into
   a half-bank bf16 PSUM tile. (XBAR dma_start_transpose is far worse:
   ~15 GB/s and the DGE doorbell blocks the issuing engine ~1.3us each.)
 - LN emission is SPLIT: stats+apply (DVE) are emitted right after their
   producer x-tiles finalize (inside the MLP W2 tail / proj tail /
   embedding loop), while the PE transposes are deferred through a pend
   queue and injected behind ready PE work (between W1 and W2 of each MLP
   n-block, before q-chunks 2/3 at attention top, before LM tiles 8/12).
   This keeps the PE FIFO head from blocking on the DVE chain at phase
   boundaries and keeps HAM near K=8/8.
 - attention: score matmuls contract over head_dim=64, so the even-head
   (PE rows 0:63) and odd-head (rows 64:127) units are issued back-to-back
   and run concurrently in distinct PE row-groups (tile_position derived
   from base_partition), halving score streaming and hiding their
   LDWEIGHTS under the other sub's matmul. Units run in 2-unit waves
   (sub0+sub1 of one sequence); exp is batched per (ki, sub) straight out
   of PSUM on ACT; the causal mask is a DVE multiply of the bf16 diagonal
   block by a 0/1 lower-tri mask; V is augmented as [ones64 | V64] so the
   softmax denominator lands in PSUM rows 0:64 (reciprocal_approx_fast
   needs partition-0 input). PSUM: gen(3)+genT(1) on psA, at(2)+ot(2) on
   psB = 8 banks.
 - engine placement is load-balance-tuned and surprisingly sensitive:
   QK-chunk copies DVE-only, V copies ACT, ln_tp/LM copies ACT, relu
   alternates ACT/DVE except ACT-only in the tail-hoisted W1 block.
   Moving proj evictions onto ACT regresses ~200us (ACT queue convoys vs
   the attention exps); GPSIMD elementwise and XBAR-DMA transposes
   regress similarly.
 - attention-tail ordering: proj tiles 0..7 are PE fillers inside pair-2
   waves; the tail emits proj(8..11), LN2-g1 stats, tp(g0), proj(12..15),
   LN2-g2 stats, tp(g1), then pends tp(g2)/tp(g3) into the MLP.
 - per-layer weight prefetch order puts the pair-0 QK weight DMAs first;
   w1all/w2all are double-buffered so their WAR on the previous MLP can't
   stall the sync DGE queue ahead of attention.
 - bf16 everywhere on the PE except the fp32 residual and the fp32r
   embedding path; LN gains/biases are folded into weights host-side.
 - do NOT try fp8 DoubleRow here: e4m3 quantization of any large matmul
   class pushes rel-err to 6e-2..1.3e-1 vs the 2e-2 gate (bf16 baseline
   noise is already 8.7e-3).
"""

import sys

for _p in ("/opt/trn_rl_repo", "/opt/pypackages"):
    if _p not in sys.path:
        sys.path.insert(0, _p)

import numpy as np
import ml_dtypes

import concourse.bass as bass
import concourse.tile as tile
from concourse import bacc, mybir
from concourse.bass_utils import run_bass_kernel_spmd

F32 = mybir.dt.float32
F32R = mybir.dt.float32r
BF16 = mybir.dt.bfloat16

N_EMBED = 384
CONTEXT = 512
N_HEADS = 6
HEAD_DIM = 64
N_LAYERS = 6
VOCAB = 65
B, T = 32, 512
LN_EPS = 1e-5
N_CORES = 8
B_LOC = B // N_CORES          # 4 sequences per core
N_TOK = B_LOC * T             # 2048 tokens per core
N_TILES = N_TOK // 128        # 16 token tiles
N_CHUNKS = N_EMBED // 128     # 3 E-chunks
N_MLP = 4 * N_EMBED           # 1536
N_MCHUNK = N_MLP // 128       # 12
SCALE = float(N_EMBED) ** -0.5
MDT = F32R
NEG_BIG = -1.0e30
V_W = N_HEADS * 128           # [ones64 | V64] per head -> 768 cols


def _prep(inputs):
    """Host-side layout prep + exact LN folds. Returns (shared, has, per_core_idx)."""
    f = lambda a: np.ascontiguousarray(np.asarray(a), dtype=np.float32)
    idx = np.asarray(inputs["idx"])
    tok_emb, pos_emb = f(inputs["tok_emb"]), f(inputs["pos_emb"])
    Wq, Wk, Wv = f(inputs["Wq"]), f(inputs["Wk"]), f(inputs["Wv"])
    Wproj, bproj = f(inputs["Wproj"]), f(inputs["bproj"])
    W1, b1, W2, b2 = f(inputs["W1"]), f(inputs["b1"]), f(inputs["W2"]), f(inputs["b2"])
    ln1_g, ln1_b = f(inputs["ln1_g"]), f(inputs["ln1_b"])
    ln2_g, ln2_b = f(inputs["ln2_g"]), f(inputs["ln2_b"])
    lnf_g, lnf_b = f(inputs["lnf_g"]), f(inputs["lnf_b"])
    Wlm, blm = f(inputs["Wlm"]), f(inputs["blm"])

    L, H, E, D = N_LAYERS, N_HEADS, N_EMBED, HEAD_DIM

    # fold ln gains into the consuming weights (exact)
    Wq_f = ln1_g[:, None, :, None] * Wq          # [L,H,E,D]
    Wk_f = ln1_g[:, None, :, None] * Wk
    Wv_f = ln1_g[:, None, :, None] * Wv
    W1_f = ln2_g[:, :, None] * W1                # [L,E,4E]
    Wlm_f = lnf_g[:, None] * Wlm                 # [E,V]

    # ln biases propagate through the matmuls as constant bias vectors
    qb = np.einsum("le,lhed->lhd", ln1_b, Wq)    # [L,H,D]
    kb = np.einsum("le,lhed->lhd", ln1_b, Wk)
    vb = np.einsum("le,lhed->lhd", ln1_b, Wv)
    b1_eff = b1 + np.einsum("le,lem->lm", ln2_b, W1)    # [L,4E]
    blm_eff = blm + lnf_b @ Wlm                          # [V]

    # head-pair packed QT/KT weights: [L, 3, E, 128]  (pair r = heads 2r, 2r+1)
    wqp = np.concatenate([Wq_f[:, 0::2], Wq_f[:, 1::2]], axis=-1)  # [L,3,E,128]
    wkp = np.concatenate([Wk_f[:, 0::2], Wk_f[:, 1::2]], axis=-1)
    qbp = np.concatenate([qb[:, 0::2], qb[:, 1::2]], axis=-1)      # [L,3,128]
    kbp = np.concatenate([kb[:, 0::2], kb[:, 1::2]], axis=-1)
    wv_all = Wv_f.transpose(0, 2, 1, 3).reshape(L, E, H * D)       # [L,E,384]
    vb_all = vb.reshape(L, H * D)

    # causal mask as additive matmul rhs: -BIG on strict lower triangle (k > j)
    trib = (np.tril(np.ones((128, 128), dtype=np.float32), -1) * NEG_BIG)

    shared = dict(
        tok_emb=tok_emb,
        pos_emb=pos_emb,
        wqp=np.ascontiguousarray(wqp.astype(ml_dtypes.bfloat16)),
        wkp=np.ascontiguousarray(wkp.astype(ml_dtypes.bfloat16)),
        wv=np.ascontiguousarray(wv_all.astype(ml_dtypes.bfloat16)),
        wp=np.ascontiguousarray(Wproj.astype(ml_dtypes.bfloat16)),
        w1=np.ascontiguousarray(W1_f.astype(ml_dtypes.bfloat16)),
        w2=np.ascontiguousarray(W2.astype(ml_dtypes.bfloat16)),
        wlm=np.ascontiguousarray(Wlm_f.astype(ml_dtypes.bfloat16)),
        ident=np.eye(128, dtype=ml_dtypes.bfloat16),
        identf=np.eye(128, dtype=np.float32),
        pos4=np.ascontiguousarray(
            pos_emb[:CONTEXT].reshape(4, 128, N_EMBED)),
        iota=np.arange(VOCAB, dtype=np.float32).reshape(VOCAB, 1),
        trib=np.ascontiguousarray(trib.astype(ml_dtypes.bfloat16)),
    )
    flags = dict(
        qb=qbp if np.any(qbp) else None,
        kb=kbp if np.any(kbp) else None,
        vb=np.broadcast_to(vb_all[:, None, :], (L, 128, H * D)).copy()
        if np.any(vb) else None,
        bp=np.broadcast_to(bproj[:, None, :], (L, 128, E)).copy()
        if np.any(bproj) else None,
        b1=np.ascontiguousarray(b1_eff.reshape(L, N_MCHUNK, 128).transpose(0, 2, 1))
        if np.any(b1_eff) else None,                    # [L,128,12] partition-major
        b2=np.broadcast_to(b2[:, None, :], (L, 128, E)).copy() if np.any(b2) else None,
        blm=np.broadcast_to(blm_eff[None, :], (128, VOCAB)).copy()
        if np.any(blm_eff) else None,
    )
    for k, v in flags.items():
        if v is not None:
            shared[k] = np.ascontiguousarray(v, dtype=np.float32)
    has = {k: (v is not None) for k, v in flags.items()}

    idx_f = idx.astype(np.float32).reshape(N_CORES, N_TOK)
    return shared, has, idx_f


def _build(has):
    nc = bacc.Bacc(trn_type="TRN2", debug=False, num_devices=N_CORES)
    d = {}
    d["idxf"] = nc.dram_tensor("idxf", [N_TOK], F32, kind="ExternalInput")
    d["tok_emb"] = nc.dram_tensor("tok_emb", [VOCAB, N_EMBED], MDT, kind="ExternalInput")
    d["pos_emb"] = nc.dram_tensor("pos_emb", [CONTEXT, N_EMBED], F32, kind="ExternalInput")
    d["wqp"] = nc.dram_tensor("wqp", [N_LAYERS, 3, N_EMBED, 128], BF16, kind="ExternalInput")
    d["wkp"] = nc.dram_tensor("wkp", [N_LAYERS, 3, N_EMBED, 128], BF16, kind="ExternalInput")
    d["wv"] = nc.dram_tensor("wv", [N_LAYERS, N_EMBED, N_EMBED], BF16, kind="ExternalInput")
    d["wp"] = nc.dram_tensor("wp", [N_LAYERS, N_EMBED, N_EMBED], BF16, kind="ExternalInput")
    d["w1"] = nc.dram_tensor("w1", [N_LAYERS, N_EMBED, N_MLP], BF16, kind="ExternalInput")
    d["w2"] = nc.dram_tensor("w2", [N_LAYERS, N_MLP, N_EMBED], BF16, kind="ExternalInput")
    d["wlm"] = nc.dram_tensor("wlm", [N_EMBED, VOCAB], BF16, kind="ExternalInput")
    d["ident"] = nc.dram_tensor("ident", [128, 128], BF16, kind="ExternalInput")
    d["identf"] = nc.dram_tensor("identf", [128, 128], MDT, kind="ExternalInput")
    d["pos4"] = nc.dram_tensor("pos4", [4, 128, N_EMBED], MDT, kind="ExternalInput")
    d["iota"] = nc.dram_tensor("iota", [VOCAB, 1], F32, kind="ExternalInput")
    d["trib"] = nc.dram_tensor("trib", [128, 128], BF16, kind="ExternalInput")
    if has["qb"]:
        d["qb"] = nc.dram_tensor("qb", [N_LAYERS, 3, 128], F32, kind="ExternalInput")
    if has["kb"]:
        d["kb"] = nc.dram_tensor("kb", [N_LAYERS, 3, 128], F32, kind="ExternalInput")
    if has["vb"]:
        d["vb"] = nc.dram_tensor("vb", [N_LAYERS, 128, N_EMBED], F32, kind="ExternalInput")
    if has["bp"]:
        d["bp"] = nc.dram_tensor("bp", [N_LAYERS, 128, N_EMBED], F32, kind="ExternalInput")
    if has["b1"]:
        d["b1"] = nc.dram_tensor("b1", [N_LAYERS, 128, N_MCHUNK], F32, kind="ExternalInput")
    if has["b2"]:
        d["b2"] = nc.dram_tensor("b2", [N_LAYERS, 128, N_EMBED], F32, kind="ExternalInput")
    if has["blm"]:
        d["blm"] = nc.dram_tensor("blm", [128, VOCAB], F32, kind="ExternalInput")
    logits_d = nc.dram_tensor("logits", [N_TOK, VOCAB], F32, kind="ExternalOutput")

    AF = mybir.ActivationFunctionType
    OP = mybir.AluOpType

    with tile.TileContext(nc) as tc:
        with tc.tile_pool(name="const", bufs=1) as cst, \
             tc.tile_pool(name="persist", bufs=1) as per, \
             tc.tile_pool(name="work", bufs=3) as wk, \
             tc.tile_pool(name="htile", bufs=4) as hp, \
             tc.tile_pool(name="wts", bufs=4) as wts, \
             tc.tile_pool(name="psA", bufs=3, space="PSUM") as psA, \
             tc.tile_pool(name="psB", bufs=2, space="PSUM") as psB:

            # ---- constants (embedding-critical DMAs first) ----
            iota = cst.tile([VOCAB, 1], F32)
            nc.sync.dma_start(iota, d["iota"][:, :])
            tok_sb = cst.tile([VOCAB, N_EMBED], MDT)
            nc.sync.dma_start(tok_sb, d["tok_emb"][:, :])
            ident = cst.tile([128, 128], BF16)
            nc.sync.dma_start(ident, d["ident"][:, :])
            identf = cst.tile([128, 128], MDT)
            nc.sync.dma_start(identf, d["identf"][:, :])
            pos_sb = cst.tile([128, 4, N_EMBED], MDT)
            nc.sync.dma_start(pos_sb, d["pos4"].rearrange("q p e -> p q e"))
            trib = cst.tile([128, 128], BF16)
            nc.sync.dma_start(trib, d["trib"][:, :])
            eps_sb = cst.tile([128, 1], F32)
            nc.vector.memset(eps_sb, LN_EPS)
            tri01 = cst.tile([128, 128], BF16)
            nc.vector.tensor_scalar(out=tri01, in0=trib, scalar1=0.0,
                                    scalar2=None, op0=OP.is_equal)

            bias_sb = {}
            for nm in ("vb", "bp", "b2"):
                if has[nm]:
                    bias_sb[nm] = cst.tile([128, N_LAYERS, N_EMBED], F32)
                    nc.sync.dma_start(bias_sb[nm], d[nm].rearrange("l p e -> p l e"))
            if has["b1"]:
                bias_sb["b1"] = cst.tile([128, N_LAYERS, N_MCHUNK], F32)
                nc.sync.dma_start(bias_sb["b1"], d["b1"].rearrange("l p m -> p l m"))
            for nm in ("qb", "kb"):
                if has[nm]:
                    bias_sb[nm] = cst.tile([128, N_LAYERS, 3], F32)
                    nc.sync.dma_start(bias_sb[nm], d[nm].rearrange("l r p -> p l r"))
            if has["blm"]:
                bias_sb["blm"] = cst.tile([128, VOCAB], F32)
                nc.sync.dma_start(bias_sb["blm"], d["blm"][:, :])

            # ---- HAM warmup: ~40 dep-free matmuls keep the PE busy from
            # t=0 so the clock gate opens at ~4us instead of ~49us (the
            # embedding phase is DMA/DVE-gated and too sparse to warm it) ----
            for _w in range(40):
                pwm = psA.tile([128, 512], F32, tag="gen", name="pwm")
                nc.tensor.matmul(pwm[:, :128], lhsT=ident, rhs=ident,
                                 start=True, stop=True)

            # ---- persistent activations ----
            x = per.tile([128, N_TILES, N_EMBED], F32)          # residual, token-major
            v_aug = per.tile([128, N_TILES, V_W], BF16)         # [ones64|V64] per head
            ones_blk = cst.tile([128, 64], F32)
            nc.vector.memset(ones_blk, 1.0)
            nc.vector.tensor_copy(
                v_aug.rearrange("p t (h j) -> p t h j", h=N_HEADS)[:, :, :, 0:64],
                ones_blk[:, None, None, :].to_broadcast(
                    [128, N_TILES, N_HEADS, 64]))

            # round-robin engine pickers for PSUM->SBUF copies
            _rr = {"c": 0, "q": 0}

            def copy_out(dst, src):
                _rr["c"] += 1
                if _rr["c"] % 2 == 0:
                    nc.scalar.copy(dst, src)
                else:
                    nc.vector.tensor_copy(dst, src)

            # =========================================================
            # LayerNorm: stats (DVE/ACT) -> bf16 apply (DVE) -> bf16 PE
            # transposes -> PSUM -> copy into hT.
            # =========================================================
            def tile_stats(t):
                    st = wk.tile([128, 6], F32, tag="bnst", bufs=8)
                    nc.vector.bn_stats(out=st, in_=x[:, t, :])
                    return st

            def ln_stats_apply(nm, tg, sts=None):
                    mv4 = wk.tile([128, 4, 2], F32, tag="mv" + nm)
                    for dt_ in range(4):
                        st = (sts[dt_] if sts is not None
                              else tile_stats(tg * 4 + dt_))
                        nc.vector.bn_aggr(out=mv4[:, dt_, :], in_=st)
                    sstd = wk.tile([128, 4], F32, tag="sstd")
                    nc.scalar.activation(out=sstd, in_=mv4[:, :, 1],
                                         func=AF.Sqrt, bias=eps_sb, scale=1.0)
                    rstd = wk.tile([128, 4], F32, tag="rstd")
                    nc.vector.reciprocal(out=rstd, in_=sstd)
                    hts = []
                    for dt_ in range(4):
                        t = tg * 4 + dt_
                        ht = hp.tile([128, N_EMBED], BF16, tag="h", bufs=12)
                        nc.vector.tensor_scalar(
                            out=ht, in0=x[:, t, :],
                            scalar1=mv4[:, dt_, 0:1],
                            scalar2=rstd[:, dt_:dt_ + 1],
                            op0=OP.subtract, op1=OP.mult)
                        hts.append(ht)
                    return hts

            def ln_tp(dst_hT, hts, tg, act=True):
                    for c in range(N_CHUNKS):
                        pt = psA.tile([128, 512], BF16, tag="genT", bufs=1,
                                      name="pt")
                        for dt_ in range(4):
                            nc.tensor.transpose(
                                pt[:, dt_ * 128:(dt_ + 1) * 128],
                                hts[dt_][:, c * 128:(c + 1) * 128], ident)
                        dst = dst_hT[:, c, tg * 512:(tg + 1) * 512]
                        if act:
                            nc.scalar.copy(dst, pt)
                        else:
                            copy_out(dst, pt)

            def new_hT():
                return per.tile([128, N_CHUNKS, N_TOK], BF16, tag="ht", bufs=2,
                                name="hT")

            # =========================================================
            # embedding: x = onehot(idx) @ tok_emb + pos; LN1 of layer 0
            # =========================================================
            hT = new_hT()
            pend_tp = []
            idx_bs = []
            for t in range(N_TILES):
                idx_b = wk.tile([VOCAB, 128], F32, tag="idxb", bufs=8,
                                name="idx_b")
                nc.sync.dma_start(
                    idx_b,
                    bass.AP(tensor=d["idxf"], offset=t * 128,
                            ap=[[0, VOCAB], [1, 128]]))
                idx_bs.append(idx_b)
            for t in range(N_TILES):
                idx_b = idx_bs[t]
                oh = wk.tile([VOCAB, 128], MDT, tag="oh")
                nc.vector.tensor_scalar(out=oh, in0=idx_b, scalar1=iota,
                                        scalar2=None, op0=OP.is_equal)
                pe = psA.tile([128, 512], F32, tag="gen")
                nc.tensor.matmul(pe[:, :N_EMBED], lhsT=oh,
                                 rhs=tok_sb, start=True, stop=False)
                nc.tensor.matmul(pe[:, :N_EMBED], lhsT=identf,
                                 rhs=pos_sb[:, t % 4, :],
                                 start=False, stop=True)
                nc.scalar.copy(x[:, t, :], pe[:, :N_EMBED])
                if t % 4 == 3:
                    if pend_tp:
                        pend_tp.pop(0)()
                    hts_g = ln_stats_apply("1", t // 4)
                    pend_tp.append(lambda hts=hts_g, g=t // 4, dst=hT:
                                   ln_tp(dst, hts, g))

            for layer in range(N_LAYERS):
                # ---- prefetch all weights of this layer (pair-0 QK weight
                # DMAs must go FIRST: w1all/w2all WAR-wait on the previous
                # MLP inside the same sync DGE queue and would stall the
                # first attention matmuls behind them) ----
                otc = per.tile([128, N_CHUNKS, N_TOK], BF16, tag="big")

                # ---- QT/KT chunk emitters ----
                def emit_qk_chunks(pair, halves=False):
                    qkt, chunks = {}, []
                    W = 256 if halves else 512
                    for nm, wd, bias_nm in (("q", d["wqp"], "qb"),
                                            ("k", d["wkp"], "kb")):
                        wqk = wts.tile([128, N_CHUNKS, 128], BF16, tag="wqk",
                                       bufs=2, name=f"wqk_{nm}")
                        for c in range(N_CHUNKS):
                            nc.sync.dma_start(
                                wqk[:, c, :],
                                wd[layer, pair, c * 128:(c + 1) * 128, :])
                        # rows 0:64 = even head (sub0), 64:128 = odd head
                        dstT = per.tile([128, N_TOK], BF16, tag="qk" + nm,
                                        bufs=2, name=f"qk_{nm}")
                        qkt[nm] = dstT

                        def chunk(n, wqk=wqk, dstT=dstT, bias_nm=bias_nm):
                            pq = psA.tile([128, 512], F32, tag="gen", name="pq")
                            for c in range(N_CHUNKS):
                                nc.tensor.matmul(
                                    pq[:, :W], lhsT=wqk[:, c, :],
                                    rhs=hT[:, c, n * W:(n + 1) * W],
                                    start=(c == 0), stop=(c == N_CHUNKS - 1))
                            dst = dstT[:, n * W:(n + 1) * W]
                            if has[bias_nm]:
                                nc.scalar.activation(
                                    out=dst, in_=pq[:, :W], func=AF.Identity,
                                    bias=bias_sb[bias_nm][:, layer, pair:pair + 1],
                                    scale=1.0)
                            else:
                                nc.vector.tensor_copy(dst, pq[:, :W])

                        for n in range(N_TOK // W):
                            chunks.append(lambda n=n, chunk=chunk: chunk(n))
                    return qkt, chunks

                # ---- V tile emitter ----
                def emit_v(t):
                    pv = psA.tile([128, 512], F32, tag="gen")
                    for c in range(N_CHUNKS):
                        nc.tensor.matmul(pv[:, :N_EMBED],
                                         lhsT=hT[:, c, t * 128:(t + 1) * 128],
                                         rhs=wv_c[c],
                                         start=(c == 0), stop=(c == N_CHUNKS - 1))
                    src = pv[:, :N_EMBED].rearrange("p (h j) -> p h j", h=N_HEADS)
                    dst = v_aug[:, t, :].rearrange(
                        "p (h j) -> p h j", h=N_HEADS)[:, :, 64:128]
                    if has["vb"]:
                        nc.vector.tensor_tensor(
                            out=dst, in0=src,
                            in1=bias_sb["vb"][:, layer, :].rearrange(
                                "p (h j) -> p h j", h=N_HEADS),
                            op=OP.add)
                    else:
                        nc.scalar.copy(dst, src)

                # ---- proj tile emitter (split=True frees the PSUM buffer
                # via a fast ACT copy; only safe outside the exp-heavy waves)
                def emit_proj(t, split=False):
                    pp = psA.tile([128, 512], F32, tag="gen")
                    for c in range(N_CHUNKS):
                        nc.tensor.matmul(pp[:, :N_EMBED],
                                         lhsT=otc[:, c, t * 128:(t + 1) * 128],
                                         rhs=wp_c[c],
                                         start=(c == 0), stop=(c == N_CHUNKS - 1))
                    src_ap = pp[:, :N_EMBED]
                    if has["bp"]:
                        tmp = hp.tile([128, N_EMBED], F32, tag="hbp")
                        nc.vector.tensor_tensor(out=tmp, in0=src_ap,
                                                in1=bias_sb["bp"][:, layer, :],
                                                op=OP.add)
                        src_ap = tmp
                    if split:
                        po = hp.tile([128, N_EMBED], F32, tag="po", bufs=4)
                        nc.scalar.copy(po, src_ap)
                        src_ap = po
                    nc.vector.tensor_tensor(out=x[:, t, :], in0=src_ap,
                                            in1=x[:, t, :], op=OP.add)

                # ---- attention ----
                qkt, chunks0 = emit_qk_chunks(0)
                wv_c, wp_c = [], []
                for c in range(N_CHUNKS):
                    w = wts.tile([128, N_EMBED], BF16, tag="wvchk", bufs=3)
                    nc.sync.dma_start(w, d["wv"][layer, c * 128:(c + 1) * 128, :])
                    wv_c.append(w)
                for c in range(N_CHUNKS):
                    w = wts.tile([128, N_EMBED], BF16, tag="wpchk", bufs=3)
                    nc.sync.dma_start(w, d["wp"][layer, c * 128:(c + 1) * 128, :])
                    wp_c.append(w)
                w1all = wts.tile([128, N_CHUNKS, N_MLP], BF16, tag="w1all", bufs=2)
                for c in range(N_CHUNKS):
                    nc.sync.dma_start(
                        w1all[:, c, :], d["w1"][layer, c * 128:(c + 1) * 128, :])
                w2all = wts.tile([128, N_MCHUNK, N_EMBED], BF16, tag="w2all", bufs=1)
                for m in range(N_MCHUNK):
                    nc.sync.dma_start(
                        w2all[:, m, :], d["w2"][layer, m * 128:(m + 1) * 128, :])
                # q0 q1 k0 k1 first; tp(ln1-g2) before q2/k2; tp(ln1-g3)
                # deferred behind 4 V tiles so its DVE apply chain (which
                # only starts after W2(3)) is fully covered by PE work
                for i in (0, 1, 4, 5):
                    chunks0[i]()
                if pend_tp:
                    pend_tp.pop(0)()
                for i in (2, 6):
                    chunks0[i]()
                for t in range(4):          # V tiles 0..3 need hT groups 0,1
                    emit_v(t)
                while pend_tp:
                    pend_tp.pop(0)()
                for i in (3, 7):
                    chunks0[i]()
                for t in range(4, 8):
                    emit_v(t)

                fillers = []

                def pop_filler():
                    if fillers:
                        fillers.pop(0)()

                for pair in range(3):
                    if pair == 0:
                        fillers = [lambda t=t: emit_v(t) for t in range(8, 16)]
                        qkt_n, chunks_n = emit_qk_chunks(1)
                        fillers += chunks_n
                    elif pair == 1:
                        qkt_n, chunks_n = emit_qk_chunks(2, halves=True)
                        fillers = list(chunks_n)
                    else:
                        fillers = []

                    # waves: one sequence s, both head-parities (sub 0 on PE
                    # rows 0:63, sub 1 on rows 64:127 -> concurrent scores
                    # into the two banks of one [128,1024] PSUM tile, so the
                    # exp is a single batched ACT op per key tile.
                    wave_order = (0, 1, 2, 3)
                    for wi, s in enumerate(wave_order):
                        pos = [psB.tile([128, 512], F32, tag="ot", bufs=2,
                                        name=f"pos_{s}_{sub}")
                               for sub in range(2)]
                        at = wk.tile([128, 4, 2, 512], BF16, tag="at_sb",
                                     bufs=2, name=f"at_{s}")
                        for ki in range(4):
                            width = 512 - ki * 128
                            kc = s * 512 + ki * 128
                            pa = [psB.tile([128, 512], F32, tag="at",
                                           name=f"pa_{s}_{ki}_{sub}")
                                  for sub in range(2)]
                            for sub in range(2):
                                nc.tensor.matmul(
                                    pa[sub][:, :width],
                                    lhsT=qkt["k"][64 * sub:64 * sub + 64,
                                                  kc:kc + 128],
                                    rhs=qkt["q"][64 * sub:64 * sub + 64,
                                                 kc:s * 512 + 512],
                                    start=True, stop=True)
                            for sub in range(2):
                                nc.scalar.activation(
                                    out=at[:, ki, sub, :width],
                                    in_=pa[sub][:, :width],
                                    func=AF.Exp, scale=SCALE)
                            nc.vector.tensor_tensor(
                                out=at[:, ki, :, 0:128],
                                in0=at[:, ki, :, 0:128],
                                in1=tri01[:, None, :].to_broadcast(
                                    [128, 2, 128]),
                                op=OP.mult)
                            pop_filler()
                            for sub in range(2):
                                h = 2 * pair + sub
                                nc.tensor.matmul(
                                    pos[sub][:, ki * 128:512],
                                    lhsT=v_aug[:, s * 4 + ki,
                                               h * 128:(h + 1) * 128],
                                    rhs=at[:, ki, sub, :width],
                                    start=(ki == 0), stop=(ki == 3))
                        # denominator is always rows 0:64 ([ones|V] layout)
                        rho = wk.tile([64, 2, 512], F32, tag="rho", bufs=2,
                                      name=f"rho_{s}")
                        for sub in range(2):
                            nc.vector.reciprocal_approx_fast(
                                out=rho[:, sub, :], in_=pos[sub][0:64, :])
                            nc.vector.tensor_tensor(
                                out=otc[64 * sub:64 * sub + 64, pair,
                                        s * 512:(s + 1) * 512],
                                in0=pos[sub][64:128, :], in1=rho[:, sub, :],
                                op=OP.mult)
                        if pair == 2 and wi == 1:
                            fillers += [lambda t=t: emit_proj(t)
                                        for t in range(8)]
                        if pair == 2 and wi == 2:
                            hts2_0 = ln_stats_apply("2", 0)
                    # drain fillers before the next pair needs its QT/KT
                    while fillers:
                        pop_filler()
                    if pair < 2:
                        qkt = qkt_n

                # ---- LN2 with split emission: stats/apply go to the DVE
                # queue early, transposes are placed behind ready PE work so
                # the PE FIFO head never waits on the DVE chain ----
                h2T = new_hT()
                mlpT = per.tile([128, N_MCHUNK, 512], BF16, tag="mlpt")

                def w1_block(n, relu_act=False):
                    for m in range(N_MCHUNK):
                        pm = psA.tile([128, 512], F32, tag="gen", name="pm")
                        for c in range(N_CHUNKS):
                            nc.tensor.matmul(
                                pm, lhsT=w1all[:, c, m * 128:(m + 1) * 128],
                                rhs=h2T[:, c, n * 512:(n + 1) * 512],
                                start=(c == 0), stop=(c == N_CHUNKS - 1))
                        if has["b1"]:
                            nc.scalar.activation(
                                out=mlpT[:, m, :], in_=pm, func=AF.Relu,
                                bias=bias_sb["b1"][:, layer, m:m + 1], scale=1.0)
                        elif relu_act or m % 2 == 0:
                            nc.scalar.activation(out=mlpT[:, m, :], in_=pm,
                                                 func=AF.Relu, scale=1.0)
                        else:
                            nc.vector.tensor_scalar(
                                out=mlpT[:, m, :], in0=pm, scalar1=0.0,
                                scalar2=None, op0=OP.max)

                for t in range(8, 11):
                    emit_proj(t)
                hts2_1 = ln_stats_apply("2", 1)
                ln_tp(h2T, hts2_0, 0)
                # W1 of n-block 0: ~8us of dep-free PE work that hides the
                # end-of-attention DVE backlog (norm(s3) + x-adds + stats);
                # proj(11) is deferred past it so its PSUM-buffer WAR on
                # xadd(8) never reaches the PE FIFO head
                w1_block(0, relu_act=True)
                for t in range(11, 16):
                    emit_proj(t)
                hts2_2 = ln_stats_apply("2", 2)
                ln_tp(h2T, hts2_1, 1)
                hts2_3 = ln_stats_apply("2", 3)
                pend_tp.append(lambda hts=hts2_2, dst=h2T: ln_tp(dst, hts, 2))
                pend_tp.append(lambda hts=hts2_3, dst=h2T: ln_tp(dst, hts, 3))

                # ---- MLP, with the next LN1 interleaved in the W2 tail ----
                last = layer == N_LAYERS - 1
                if last:
                    hT = per.tile([128, N_CHUNKS, N_TOK], BF16, tag="xf")
                    wlm_c = []
                    for c in range(N_CHUNKS):
                        w = wts.tile([128, VOCAB], BF16, tag="wlm", bufs=3)
                        nc.sync.dma_start(w, d["wlm"][c * 128:(c + 1) * 128, :])
                        wlm_c.append(w)
                    nm = "f"
                else:
                    hT = new_hT()
                    nm = "1"
                for n in range(N_TOK // 512):
                    if n > 0:
                        w1_block(n)
                    if pend_tp:
                        pend_tp.pop(0)()
                    sts = []
                    for dt in range(4):
                        t = n * 4 + dt
                        pw = psA.tile([128, 512], F32, tag="gen")
                        for m in range(N_MCHUNK):
                            nc.tensor.matmul(
                                pw[:, :N_EMBED],
                                lhsT=mlpT[:, m, dt * 128:(dt + 1) * 128],
                                rhs=w2all[:, m, :],
                                start=(m == 0), stop=(m == N_MCHUNK - 1))
                        if has["b2"]:
                            tmp = hp.tile([128, N_EMBED], F32, tag="hbp")
                            nc.vector.tensor_tensor(out=tmp, in0=pw[:, :N_EMBED],
                                                    in1=bias_sb["b2"][:, layer, :],
                                                    op=OP.add)
                            nc.vector.tensor_tensor(out=x[:, t, :], in0=tmp,
                                                    in1=x[:, t, :], op=OP.add)
                        else:
                            nc.vector.tensor_tensor(out=x[:, t, :],
                                                    in0=pw[:, :N_EMBED],
                                                    in1=x[:, t, :], op=OP.add)
                        sts.append(tile_stats(t))
                    hts_g = ln_stats_apply(nm, n, sts=sts)
                    pend_tp.append(lambda hts=hts_g, g=n, dst=hT:
                                   ln_tp(dst, hts, g))

            # ---- LM head (final LN already produced hT under tag "xf") ----
            for t in range(N_TILES):
                if t in (8, 12) and pend_tp:
                    pend_tp.pop(0)()
                pl = psA.tile([128, 512], F32, tag="gen")
                for c in range(N_CHUNKS):
                    nc.tensor.matmul(pl[:, :VOCAB],
                                     lhsT=hT[:, c, t * 128:(t + 1) * 128],
                                     rhs=wlm_c[c],
                                     start=(c == 0), stop=(c == N_CHUNKS - 1))
                lg = wk.tile([128, VOCAB], F32, tag="lg")
                if has["blm"]:
                    nc.vector.tensor_tensor(out=lg, in0=pl[:, :VOCAB],
                                            in1=bias_sb["blm"], op=OP.add)
                else:
                    nc.scalar.copy(lg, pl[:, :VOCAB])
                nc.sync.dma_start(logits_d[t * 128:(t + 1) * 128, :], lg)

    nc.compile()
    return nc


_CACHE = {}


def _get_nc(has):
    key = tuple(sorted(has.items()))
    if key not in _CACHE:
        _CACHE[key] = _build(has)
    return _CACHE[key]


def kernel(**inputs):
    shared, has, idx_f = _prep(inputs)
    nc = _get_nc(has)
    in_maps = []
    for core in range(N_CORES):
        m = dict(shared)
        m["idxf"] = idx_f[core]
        in_maps.append(m)
    res = run_bass_kernel_spmd(nc, in_maps, core_ids=list(range(N_CORES)))
    out = np.stack([r["logits"].reshape(B_LOC, T, VOCAB) for r in res.results])
    return out.reshape(B, T, VOCAB)
